# revision 37
# baseline (speedup 1.0000x reference)
"""MultiHeadAttention (cross-attention, B=32 N=512 L=1024 D=512 H=8) on 8 TRN2 cores.

Data parallel (4 batches/core). Host prep: per-batch gather of unmasked K/V
positions (counts ~512 of 1024), batches sorted by count and dealt to cores so
each program slot gets a uniform l-chunk count (seed-0 data -> (5,5,5,4)).

Q/K/V projections run as fp8e4m3 DoubleRow matmuls with 3-term hi/lo error
compensation (x_hi*W_hi + x_lo*W_hi + x_hi*W_lo), where x_hi/x_lo and
16*W hi/lo splits are precomputed on host. DoubleRow contracts 2 k-tiles of
128 per instruction at 0.5 cycles/row -> projections cost 1536 cycles per
128x512 output vs 2048 in bf16, with bf16-level accuracy. The x16 weight
scale is folded into the exp scale (q,k both x16 -> exp scale = SCALE/256)
and the V ones-column (16.0 -> reciprocal absorbs the scale).

Device (S/PV/o_proj matmuls bf16, psum f32):
  - scores S^T[l,n]: head-pairs packed in one [128,2N] psum via tile_position
    (two 64-channel heads in PE row groups 0-63/64-127)
  - exp on ACT with per-partition mask bias (host-compacted -87 pad bias)
  - PV stationary-swap: exp'd scores as the stationary operand [l,n-chunk],
    [V|16] as a 65-column moving operand -> O[n, c|16den] with the softmax
    denominator landing per-partition; normalize = reciprocal +
    tensor_scalar_mul (no partition broadcast); psum pre-zeroed by DVE memset
    so accumulation groups never use start=True (avoids whole-tile WAR)
  - O transposed back by PE in bf16 (1 cycle/row vs 2 for f32) for the
    o_proj stationary; psum->sbuf copy gets the DVE 2x 16-bit mode
  - o_proj bias folded into the psum accumulation as a 1-partition matmul
    (ones x bo) for the last batch; DVE add for batches hidden in the pipeline
  - PE warmup matmuls at t=0 hold the p-state ramp while input DMAs stream
  - software pipeline: batch b+1's projections and batch b-1's o_proj run as
    fillers inside batch b's attention pairs; out-DMAs ride the idle SP queue
    so they never block the ACT exp stream
"""
import sys

sys.path.insert(0, "/opt/trn_rl_repo")
import numpy as np

B, N, L, D, H = 32, 512, 1024, 512, 8
C = D // H
NCORES = 8
BLOC = B // NCORES
SCALE = C ** -0.5
SSCALE = SCALE / 256.0  # q,k both carry x16 from the fp8 weight scaling
MASK_NEG = -87.0
P = 128
NDC = D // P   # 4 d/e chunks
NNC = N // P   # 4 n chunks
LCMAX = L
NSLOT = 8      # fp8 k-tile slots: 0-3 = hi d-chunks, 4-7 = lo d-chunks
# 3-term hi/lo: (x slot base, w slot base) per term
TERMS = ((0, 0), (4, 0), (0, 4))

_CACHE = {}
N_WARM = 30


def _spans(w):
    # moving-operand output spans <=512
    return [(s, min(s + 512, w)) for s in range(0, w, 512)]


def _build_nc(chunks):
    import concourse.bacc as bacc
    import concourse.tile as tile
    from concourse import mybir

    f32 = mybir.dt.float32
    bf16 = mybir.dt.bfloat16
    fp8 = mybir.dt.float8e4
    DR = mybir.MatmulPerfMode.DoubleRow
    EXP = mybir.ActivationFunctionType.Exp
    cmax = max(chunks)

    nc = bacc.Bacc()
    labels = _CACHE.setdefault(("labels", chunks), [])
    labels.clear()

    def _lab(s):
        labels.append(s)
    xq8_d = nc.declare_dram_parameter("xq8", [BLOC, NSLOT, P, N], fp8,
                                      isOutput=False)
    xk8_d = nc.declare_dram_parameter("xk8", [BLOC, NSLOT, P, LCMAX], fp8,
                                      isOutput=False)
    rpbT_d = nc.declare_dram_parameter("rpbT", [BLOC, D, LCMAX], bf16,
                                       isOutput=False)
    mb_d = nc.declare_dram_parameter("mbias", [BLOC, LCMAX], f32,
                                     isOutput=False)
    Wq8 = nc.declare_dram_parameter("Wq8", [NSLOT, P, D], fp8, isOutput=False)
    Wk8 = nc.declare_dram_parameter("Wk8", [NSLOT, P, D], fp8, isOutput=False)
    Wv8 = nc.declare_dram_parameter("Wv8", [NSLOT, P, D], fp8, isOutput=False)
    Wo = nc.declare_dram_parameter("Wo", [D, D], bf16, isOutput=False)
    bo = nc.declare_dram_parameter("bo", [1, D], bf16, isOutput=False)
    id_d = nc.declare_dram_parameter("ident", [P, P], bf16, isOutput=False)
    out = nc.declare_dram_parameter("out", [BLOC, N, D], bf16, isOutput=True)

    with tile.TileContext(nc) as tc:
        with (
            tc.tile_pool(name="consts", bufs=1) as consts,
            tc.tile_pool(name="xin", bufs=2) as xin_pool,
            tc.tile_pool(name="qk", bufs=2) as qk_pool,
            tc.tile_pool(name="vp", bufs=2) as vp_pool,
            tc.tile_pool(name="pt", bufs=2) as pt_pool,
            tc.tile_pool(name="onm", bufs=2) as onm_pool,
            tc.tile_pool(name="otp", bufs=3) as ot_pool,
            tc.tile_pool(name="outst", bufs=4) as outst_pool,
            tc.tile_pool(name="small", bufs=4) as small,
            tc.tile_pool(name="ps_sc", bufs=2, space="PSUM") as ps_sc,
            tc.tile_pool(name="ps_po", bufs=2, space="PSUM") as ps_po,
            tc.tile_pool(name="ps_mm", bufs=2, space="PSUM") as ps_mm,
        ):
            state = {}

            # ---- warmups: hold PE busy from t=0 so the p-state ramp is hot
            # by the time real operands arrive ----
            wz = nc.const_aps.tensor(1.0, (P, 256), bf16)
            for w in range(N_WARM):
                wm = ps_mm.tile([P, 512], f32, tag="mm", name="wm")
                _lab("warm"); nc.tensor.matmul(wm[:, 0:256], wz[:, 0:128], wz, start=True, stop=True)

            warm = consts.tile([P, 1], f32, tag="warm", name="warm")
            nc.vector.memset(warm, 0.0)
            nc.scalar.activation(out=warm, in_=warm, func=EXP, scale=1.0)

            ones16 = consts.tile([P, H], bf16, tag="ones16", name="ones16")
            nc.vector.memset(ones16, 16.0)

            walls = {}

            def load_w(wi, W):
                wt = consts.tile([P, NSLOT, D], fp8, tag=f"w{wi}",
                                 name=f"w{wi}")
                nc.sync.dma_start(
                    out=wt, in_=W[:].rearrange("s p e -> p s e"))
                walls[wi] = wt

            bo_row = consts.tile([1, D], bf16, tag="bo_row", name="bo_row")
            ones1 = consts.tile([1, P], bf16, tag="ones1", name="ones1")
            nc.vector.memset(ones1, 1.0)
            bo_bc = consts.tile([P, D], bf16, tag="bo_bc", name="bo_bc")

            wo_t = consts.tile([P, NDC, D], bf16, tag="wo", name="wo")

            # ---- DMA emitters ----
            def dma_xq(b):
                t = xin_pool.tile([P, NSLOT, N], fp8, tag="xq8", name="xq8")
                nc.sync.dma_start(
                    out=t, in_=xq8_d[b].rearrange("s p n -> p s n"))
                state[(b, "xq")] = t

            def dma_xk(b):
                lc = chunks[b] * P
                t = xin_pool.tile([P, NSLOT, lc], fp8, tag="xk8", name="xk8")
                nc.sync.dma_start(
                    out=t,
                    in_=xk8_d[b, :, :, 0:lc].rearrange("s p l -> p s l"))
                mbt = small.tile([P, chunks[b]], f32, tag="mbias", name="mbt")
                nc.sync.dma_start(
                    out=mbt, in_=mb_d[b, 0:lc].rearrange("(i p) -> p i", p=P))
                r = xin_pool.tile([P, NDC, lc], bf16, tag="rpbT", name="rpbT")
                nc.sync.dma_start(
                    out=r,
                    in_=rpbT_d[b, :, 0:lc].rearrange("(k p) l -> p k l", p=P))
                state[(b, "xk")] = t
                state[(b, "rpb")] = r
                state[(b, "mb")] = mbt

            # ---- prep compute groups (filler units) ----
            # fp8 DoubleRow 3-term projection: 6 insts per 128-col stationary
            def _proj3(dst, wt, wcols, xt, xcols, lab):
                k = 0
                for (xs, ws) in TERMS:
                    for s in range(2):
                        _lab(lab)
                        nc.tensor.matmul(
                            dst, wt[:, ws + 2 * s:ws + 2 * s + 2, wcols],
                            xt[:, xs + 2 * s:xs + 2 * s + 2, xcols],
                            start=(k == 0), stop=(k == 5), perf_mode=DR)
                        k += 1

            def qp_group(b, j):
                xq = state[(b, "xq")]
                pq = ps_mm.tile([P, N], f32, tag="mm", name="pq")
                _proj3(pq, walls[0], slice(j * P, (j + 1) * P),
                       xq, slice(0, N), f"QP b{b} j{j}")
                qt = qk_pool.tile([P, N], bf16, tag=f"qT{j}", name=f"qT{j}")
                nc.vector.tensor_copy(qt, pq)
                state.setdefault((b, "qT"), [None] * NDC)[j] = qt

            def kp_group(b, j, raw=False):
                lc = chunks[b] * P
                xk = state[(b, "xk")]
                rp = state[(b, "rpb")]
                dst = (qk_pool.tile([P, lc], bf16, tag=f"kraw{j}", name=f"kraw{j}")
                       if raw else
                       qk_pool.tile([P, lc], bf16, tag=f"kT{j}", name=f"kT{j}"))
                for (s0, s1) in _spans(lc):
                    pk = ps_mm.tile([P, N], f32, tag="mm", name="pk")
                    _proj3(pk[:, 0:s1 - s0], walls[1],
                           slice(j * P, (j + 1) * P),
                           xk, slice(s0, s1), f"KP b{b} j{j}")
                    if raw:
                        nc.vector.tensor_copy(dst[:, s0:s1], pk[:, 0:s1 - s0])
                    else:
                        nc.vector.tensor_add(dst[:, s0:s1], pk[:, 0:s1 - s0],
                                             rp[:, j, s0:s1])
                if raw:
                    state.setdefault((b, "kraw"), [None] * NDC)[j] = dst
                else:
                    state.setdefault((b, "kT"), [None] * NDC)[j] = dst

            def k_add(b, j):
                lc = chunks[b] * P
                rp = state[(b, "rpb")]
                kraw = state[(b, "kraw")][j]
                kt = qk_pool.tile([P, lc], bf16, tag=f"kT{j}", name=f"kT{j}")
                nc.vector.tensor_add(kt, kraw, rp[:, j, :])
                state.setdefault((b, "kT"), [None] * NDC)[j] = kt

            def vp_group(b, i):
                xk = state[(b, "xk")]
                pv = ps_mm.tile([P, N], f32, tag="mm", name="pv")
                _proj3(pv, xk, slice(i * P, (i + 1) * P),
                       walls[2], slice(0, D), f"VP b{b} i{i}")
                t = vp_pool.tile([P, H, C + 1], bf16, tag=f"vp{i}",
                                 name=f"vp{i}")
                nc.vector.tensor_copy(
                    t[:, :, 0:C], pv.rearrange("p (h c) -> p h c", h=H))
                nc.gpsimd.tensor_copy(t[:, :, C:C + 1], ones16[:, :, None])
                state.setdefault((b, "vP"), [None] * cmax)[i] = t

            def op_group(b, m):
                oT = state[(b, "oT")]
                pf = ps_mm.tile([P, N], f32, tag="mm", name="pf")
                for j in range(NDC):
                    _lab(f"OP b{b} m{m}"); nc.tensor.matmul(pf, oT[j][:, m * P:(m + 1) * P],
                                     wo_t[:, j, :],
                                     start=(j == 0), stop=(j == NDC - 1))
                to = outst_pool.tile([P, D], bf16, tag="outst", name="to")
                nc.vector.tensor_add(to, pf, bo_bc)
                nc.sync.dma_start(out=out[b, m * P:(m + 1) * P, :], in_=to)

            # last batch: accumulate the j<3 o_proj terms + bias into the sc
            # psum ring (dead after the last exp) while pair (last, j3)
            # winds down, leaving only the j3 term + copy + DMA for the tail
            parts = {}

            def op_part(b, m):
                if m % 2 == 0:
                    pw = ps_sc.tile([P, 2 * N], f32, tag="sc", name="pw")
                    parts[m] = pw[:, 0:N]
                    parts[m + 1] = pw[:, N:2 * N]
                pp = parts[m]
                oT = state[(b, "oT")]
                for j in range(NDC - 1):
                    _lab(f"OPp b{b} m{m}"); nc.tensor.matmul(pp, oT[j][:, m * P:(m + 1) * P],
                                     wo_t[:, j, :],
                                     start=(j == 0), stop=False,
                                     skip_group_check=True)
                _lab(f"OPb b{b} m{m}")
                nc.tensor.matmul(pp, ones1, bo_row, start=False, stop=False,
                                 skip_group_check=True)

            # ---- attention pair ----
            def pair(b, j, fillers, tail_fillers=()):
                c = chunks[b]
                last = b == BLOC - 1 and j == NDC - 1
                mbt = state[(b, "mb")]
                qT, kT = state[(b, "qT")], state[(b, "kT")]
                vP = state[(b, "vP")]
                po = {}
                for half in range(2):
                    po[half] = ps_po.tile([P, NNC, C + 1], f32, tag="po",
                                          name="po")
                pes = []
                fi = 0

                # PV interleaved into the S/exp stream: as soon as exp(i) is
                # done, its l-chunk is accumulated into every (m, half) psum
                # region, so PV never waits for the full exp stream.
                def pv_i(i, on_m=None):
                    pe = pes[i]
                    for m in range(NNC):
                        for half in range(2):
                            stat = pe[:, half * N + m * P:
                                       half * N + (m + 1) * P]
                            _lab(f"PV b{b} j{j} i{i} m{m}")
                            nc.tensor.matmul(
                                po[half][:, m, :], stat,
                                vP[i][:, 2 * j + half, :],
                                # first write per po bank: start=True marks
                                # the whole 2KB bank zero-on-write, replacing
                                # an explicit DVE memset
                                start=(i == 0 and m == 0),
                                stop=(i == c - 1),
                                skip_group_check=True)
                        if on_m is not None:
                            on_m(m)

                def s_exp(i):
                    pss = ps_sc.tile([P, 2 * N], f32, tag="sc", name="pss")
                    for half in range(2):
                        lo = C * half
                        _lab(f"S b{b} j{j} i{i}")
                        nc.tensor.matmul(
                            pss[:, half * N:(half + 1) * N],
                            kT[j][lo:lo + C, i * P:(i + 1) * P],
                            qT[j][lo:lo + C, :], start=True, stop=True,
                            tile_position=(lo, 0))
                    pe = pt_pool.tile([P, 2 * N], bf16, tag=f"pe{i}",
                                      name=f"pe{i}")
                    nc.scalar.activation(out=pe, in_=pss, func=EXP,
                                         bias=mbt[:, i:i + 1], scale=SSCALE)
                    pes.append(pe)

                # lag PV two rounds behind S/exp so the ACT stream is never
                # awaited (exp(i-2) is safely done when pv_i(i-2) issues)
                for i in range(c):
                    s_exp(i)
                    if i >= 1:
                        if i >= 3:
                            pv_i(i - 3)
                        if fi < len(fillers):
                            fillers[fi]()
                            fi += 1
                for k in (3, 2):
                    if c >= k:
                        pv_i(c - k)

                tr = ps_mm.tile([P, N], bf16, tag="mm", name="tr")
                onm = []

                def norm_m(m):
                    o = onm_pool.tile([P, P], bf16, tag=f"on{m}",
                                      name=f"on{m}")
                    for half in range(2):
                        r = small.tile([P, 1], f32, tag=f"rcp{half}",
                                       name=f"r{half}")
                        nc.vector.reciprocal(r, po[half][:, m, C:C + 1])
                        nc.vector.tensor_scalar_mul(
                            o[:, half * C:(half + 1) * C],
                            po[half][:, m, 0:C], r)
                    onm.append(o)

                # leftover fillers buy time for exp(c-1) to land before the
                # final PV round needs it
                while fi < len(fillers):
                    fillers[fi]()
                    fi += 1
                ot = ot_pool.tile([P, N], bf16, tag=f"oT{j}", name=f"oT{j}")
                if last:
                    # wind-down: the whole per-m output chain (o_proj partial
                    # -> transpose -> ot slice -> j3 term -> copy -> DMA)
                    # fires as each m region closes; the j3 term trails one m
                    # behind so the ot-slice latency hides under the next m's
                    # partial o_proj
                    def opf_m(m):
                        sl = slice(m * P, (m + 1) * P)
                        _lab(f"OPf b{b} m{m}")
                        nc.tensor.matmul(parts[m], ot[:, sl],
                                         wo_t[:, NDC - 1, :], start=False,
                                         stop=True, skip_group_check=True)
                        to = outst_pool.tile([P, D], bf16, tag="outst2",
                                             name="to2")
                        if m % 2 == 0:
                            nc.scalar.copy(to, parts[m])
                        else:
                            nc.vector.tensor_copy(to, parts[m])
                        nc.sync.dma_start(out=out[b, m * P:(m + 1) * P, :],
                                          in_=to)

                    def final_m(m):
                        norm_m(m)
                        op_part(b, m)
                        _lab(f"T b{b} j{j} m{m}")
                        nc.tensor.transpose(
                            tr[:, m * P:(m + 1) * P], onm[m], ident)
                        sl = slice(m * P, (m + 1) * P)
                        if m % 2 == 0:
                            nc.scalar.copy(ot[:, sl], tr[:, sl])
                        else:
                            nc.vector.tensor_copy(ot[:, sl], tr[:, sl])
                        if m >= 1:
                            opf_m(m - 1)
                    pv_i(c - 1, on_m=final_m)
                    opf_m(NNC - 1)
                else:
                    pv_i(c - 1)
                    # batched reciprocal: one strided [P, NNC] op per half,
                    # then all muls back-to-back -> T-block starts sooner
                    rs = {}
                    for half in range(2):
                        ra = small.tile([P, NNC], f32, tag=f"rcA{half}",
                                        name=f"ra{half}")
                        nc.vector.reciprocal(ra, po[half][:, :, C])
                        rs[half] = ra
                    for m in range(NNC):
                        o = onm_pool.tile([P, P], bf16, tag=f"on{m}",
                                          name=f"on{m}")
                        for half in range(2):
                            nc.vector.tensor_scalar_mul(
                                o[:, half * C:(half + 1) * C],
                                po[half][:, m, 0:C], rs[half][:, m:m + 1])
                        onm.append(o)
                    for m in range(NNC):
                        _lab(f"T b{b} j{j} m{m}")
                        nc.tensor.transpose(
                            tr[:, m * P:(m + 1) * P], onm[m], ident)
                    nc.vector.tensor_copy(ot, tr)
                state.setdefault((b, "oT"), [None] * NDC)[j] = ot

            # ---- schedule ----
            ident = consts.tile([P, P], bf16, tag="ident", name="ident")
            load_w(0, Wq8)
            dma_xq(0)
            load_w(1, Wk8)
            # xk + mb first, Wv before rpb so VP unblocks early, rpb gated last
            lc0 = chunks[0] * P
            t0 = xin_pool.tile([P, NSLOT, lc0], fp8, tag="xk8", name="xk8")
            nc.sync.dma_start(
                out=t0, in_=xk8_d[0, :, :, 0:lc0].rearrange("s p l -> p s l"))
            mbt0 = small.tile([P, chunks[0]], f32, tag="mbias", name="mbt")
            nc.sync.dma_start(
                out=mbt0, in_=mb_d[0, 0:lc0].rearrange("(i p) -> p i", p=P))
            load_w(2, Wv8)
            r0 = xin_pool.tile([P, NDC, lc0], bf16, tag="rpbT", name="rpbT")
            for k in range(NDC):
                nc.sync.dma_start(
                    out=r0[:, k, :], in_=rpbT_d[0, k * P:(k + 1) * P, 0:lc0])
            state[(0, "xk")] = t0
            state[(0, "rpb")] = r0
            state[(0, "mb")] = mbt0
            nc.sync.dma_start(
                out=wo_t, in_=Wo[:].rearrange("(k p) e -> p k e", p=P))
            nc.sync.dma_start(out=ident, in_=id_d[:])
            nc.sync.dma_start(out=bo_row, in_=bo[:])
            nc.gpsimd.partition_broadcast(bo_bc, bo_row[0:1, :], channels=P)
            for j in range(NDC):
                qp_group(0, j)
            for j in range(NDC):
                kp_group(0, j, raw=True)
            vp_group(0, 0)
            vp_group(0, 1)
            for j in range(NDC):
                k_add(0, j)

            def F(fn, *a):
                return lambda: fn(*a)

            for b in range(BLOC):
                nxt = b + 1 < BLOC
                for j in range(NDC):
                    fillers = []
                    if j == 0:
                        fillers += [F(vp_group, b, i)
                                    for i in range(2, chunks[b])]
                        if b > 0:
                            fillers.append(F(kp_group, b, 2))
                        if b > 1:
                            fillers.append(F(op_group, b - 2, 2))
                        if nxt:
                            dma_xq(b + 1)
                            dma_xk(b + 1)
                    elif j == 1:
                        if b > 0:
                            fillers.append(F(kp_group, b, 3))
                            fillers.append(F(op_group, b - 1, 0))
                        if b > 1:
                            fillers.append(F(op_group, b - 2, 3))
                        if nxt:
                            fillers += [F(qp_group, b + 1, 0),
                                        F(qp_group, b + 1, 1)]
                    elif j == 2:
                        if nxt:
                            fillers += [F(qp_group, b + 1, 2),
                                        F(qp_group, b + 1, 3),
                                        F(kp_group, b + 1, 0),
                                        F(kp_group, b + 1, 1)]
                        if b > 0:
                            fillers.append(F(op_group, b - 1, 1))
                        if not nxt:
                            fillers.append(F(op_group, b - 1, 2))
                    elif j == 3:
                        if nxt:
                            fillers += [F(vp_group, b + 1, 0),
                                        F(vp_group, b + 1, 1)]
                    if not nxt and j == 3:
                        fillers.append(F(op_group, b - 1, 3))
                    pair(b, j, fillers)

    nc.compile()
    return nc


def _get_nc(chunks=(4, 5, 5, 5)):
    chunks = tuple(chunks)
    if chunks not in _CACHE:
        _CACHE[chunks] = _build_nc(chunks)
    return _CACHE[chunks]


def _split8(a, e4):
    """f32 array -> (hi, lo) fp8 e4m3 pair with hi + lo ~= a."""
    hi = a.astype(e4)
    lo = (a - hi.astype(np.float32)).astype(e4)
    return hi, lo


def kernel(x_q, x_kv, pad_mask, Wq, Wk, Wv, Wo, bo, rpb):
    from concourse.bass_utils import run_bass_kernel_spmd
    import ml_dtypes

    bf = ml_dtypes.bfloat16
    e4 = ml_dtypes.float8_e4m3fn
    x_q = np.asarray(x_q, dtype=np.float32)
    x_kv = np.asarray(x_kv, dtype=np.float32)
    pad_mask = np.asarray(pad_mask).astype(bool)
    rpb2 = np.asarray(rpb, np.float32).reshape(L, D)

    counts = (~pad_mask).sum(axis=1)
    # ascending: the smallest batch goes first (startup is DMA-gated, so a
    # cheap batch 0 wastes less PE) and big batches fill the tail
    order = np.argsort(counts, kind="stable")  # rank -> global batch
    # slot s of core c processes global batch order[s*NCORES + c]
    chunks = []
    for s in range(BLOC):
        grp = counts[order[s * NCORES:(s + 1) * NCORES]]
        chunks.append(max(1, -(-int(grp.max()) // P)))
    chunks = tuple(chunks)
    nc = _get_nc(chunks)

    def wsplit(W):
        W16 = 16.0 * np.asarray(W, np.float32)
        hi, lo = _split8(W16, e4)
        o = np.zeros((NSLOT, P, D), e4)
        for c in range(NDC):
            o[c] = hi[c * P:(c + 1) * P]
            o[NDC + c] = lo[c * P:(c + 1) * P]
        return o

    shared = {
        "Wq8": wsplit(Wq),
        "Wk8": wsplit(Wk),
        "Wv8": wsplit(Wv),
        "Wo": np.asarray(Wo, np.float32).astype(bf),
        "bo": np.asarray(bo, np.float32).reshape(1, D).astype(bf),
        "ident": np.eye(P, dtype=np.float32).astype(bf),
    }
    in_maps = []
    for c in range(NCORES):
        xq8 = np.zeros((BLOC, NSLOT, P, N), e4)
        xk8 = np.zeros((BLOC, NSLOT, P, LCMAX), e4)
        rpbT = np.zeros((BLOC, D, LCMAX), bf)
        mb = np.full((BLOC, LCMAX), MASK_NEG, np.float32)
        for s in range(BLOC):
            g = order[s * NCORES + c]
            idx = np.nonzero(~pad_mask[g])[0]
            cnt = len(idx)
            xqT = x_q[g].T  # [D, N] f32
            hi, lo = _split8(xqT, e4)
            for k in range(NDC):
                xq8[s, k] = hi[k * P:(k + 1) * P]
                xq8[s, NDC + k] = lo[k * P:(k + 1) * P]
            xkT = x_kv[g, idx, :].T  # [D, cnt] f32
            hi, lo = _split8(xkT, e4)
            for k in range(NDC):
                xk8[s, k, :, :cnt] = hi[k * P:(k + 1) * P]
                xk8[s, NDC + k, :, :cnt] = lo[k * P:(k + 1) * P]
            rpbT[s, :, :cnt] = (16.0 * rpb2[idx, :].T).astype(bf)
            mb[s, :cnt] = 0.0
        in_maps.append({
            "xq8": xq8, "xk8": xk8, "rpbT": rpbT, "mbias": mb, **shared,
        })
    res = run_bass_kernel_spmd(nc, in_maps, list(range(NCORES)))
    outp = np.empty((B, N, D), np.float32)
    for c in range(NCORES):
        for s in range(BLOC):
            outp[order[s * NCORES + c]] = np.asarray(
                res.results[c]["out"][s], dtype=np.float32)
    return outp


# revision 38
# speedup vs baseline: 1.0106x; 1.0106x over previous
"""MultiHeadAttention (cross-attention, B=32 N=512 L=1024 D=512 H=8) on 8 TRN2 cores.

Data parallel (4 batches/core). Host prep: per-batch gather of unmasked K/V
positions (counts ~512 of 1024), batches sorted by count and dealt to cores so
each program slot gets a uniform l-chunk count (seed-0 data -> (5,5,5,4)).

Q/K/V projections run as fp8e4m3 DoubleRow matmuls with 3-term hi/lo error
compensation (x_hi*W_hi + x_lo*W_hi + x_hi*W_lo), where x_hi/x_lo and
16*W hi/lo splits are precomputed on host. DoubleRow contracts 2 k-tiles of
128 per instruction at 0.5 cycles/row -> projections cost 1536 cycles per
128x512 output vs 2048 in bf16, with bf16-level accuracy. The x16 weight
scale is folded into the exp scale (q,k both x16 -> exp scale = SCALE/256)
and the V ones-column (16.0 -> reciprocal absorbs the scale).

Device (S/PV/o_proj matmuls bf16, psum f32):
  - scores S^T[l,n]: head-pairs packed in one [128,2N] psum via tile_position
    (two 64-channel heads in PE row groups 0-63/64-127)
  - exp on ACT with per-partition mask bias (host-compacted -87 pad bias)
  - PV stationary-swap: exp'd scores as the stationary operand [l,n-chunk],
    [V|16] as a 65-column moving operand -> O[n, c|16den] with the softmax
    denominator landing per-partition; normalize = reciprocal +
    tensor_scalar_mul (no partition broadcast); psum pre-zeroed by DVE memset
    so accumulation groups never use start=True (avoids whole-tile WAR)
  - O transposed back by PE in bf16 (1 cycle/row vs 2 for f32) for the
    o_proj stationary; psum->sbuf copy gets the DVE 2x 16-bit mode
  - o_proj bias folded into the psum accumulation as a 1-partition matmul
    (ones x bo) for the last batch; DVE add for batches hidden in the pipeline
  - PE warmup matmuls at t=0 hold the p-state ramp while input DMAs stream
  - software pipeline: batch b+1's projections and batch b-1's o_proj run as
    fillers inside batch b's attention pairs; out-DMAs ride the idle SP queue
    so they never block the ACT exp stream
"""
import sys

sys.path.insert(0, "/opt/trn_rl_repo")
import numpy as np

B, N, L, D, H = 32, 512, 1024, 512, 8
C = D // H
NCORES = 8
BLOC = B // NCORES
SCALE = C ** -0.5
SSCALE = SCALE / 256.0  # q,k both carry x16 from the fp8 weight scaling
MASK_NEG = -87.0
P = 128
NDC = D // P   # 4 d/e chunks
NNC = N // P   # 4 n chunks
LCMAX = L
NSLOT = 8      # fp8 k-tile slots: 0-3 = hi d-chunks, 4-7 = lo d-chunks
# 3-term hi/lo: (x slot base, w slot base) per term
TERMS = ((0, 0), (4, 0), (0, 4))

_CACHE = {}
N_WARM = 30


def _spans(w):
    # moving-operand output spans <=512
    return [(s, min(s + 512, w)) for s in range(0, w, 512)]


def _build_nc(chunks):
    import concourse.bacc as bacc
    import concourse.tile as tile
    from concourse import mybir

    f32 = mybir.dt.float32
    bf16 = mybir.dt.bfloat16
    fp8 = mybir.dt.float8e4
    DR = mybir.MatmulPerfMode.DoubleRow
    EXP = mybir.ActivationFunctionType.Exp
    cmax = max(chunks)

    nc = bacc.Bacc()
    labels = _CACHE.setdefault(("labels", chunks), [])
    labels.clear()

    def _lab(s):
        labels.append(s)
    xq8_d = nc.declare_dram_parameter("xq8", [BLOC, NSLOT, P, N], fp8,
                                      isOutput=False)
    xk8_d = nc.declare_dram_parameter("xk8", [BLOC, NSLOT, P, LCMAX], fp8,
                                      isOutput=False)
    rpbT_d = nc.declare_dram_parameter("rpbT", [BLOC, D, LCMAX], bf16,
                                       isOutput=False)
    mb_d = nc.declare_dram_parameter("mbias", [BLOC, LCMAX], f32,
                                     isOutput=False)
    Wq8 = nc.declare_dram_parameter("Wq8", [NSLOT, P, D], fp8, isOutput=False)
    Wk8 = nc.declare_dram_parameter("Wk8", [NSLOT, P, D], fp8, isOutput=False)
    Wv8 = nc.declare_dram_parameter("Wv8", [NSLOT, P, D], fp8, isOutput=False)
    Wo = nc.declare_dram_parameter("Wo", [D, D], bf16, isOutput=False)
    bo = nc.declare_dram_parameter("bo", [1, D], bf16, isOutput=False)
    id_d = nc.declare_dram_parameter("ident", [P, P], bf16, isOutput=False)
    out = nc.declare_dram_parameter("out", [BLOC, N, D], bf16, isOutput=True)

    with tile.TileContext(nc) as tc:
        with (
            tc.tile_pool(name="consts", bufs=1) as consts,
            tc.tile_pool(name="xin", bufs=2) as xin_pool,
            tc.tile_pool(name="qk", bufs=2) as qk_pool,
            tc.tile_pool(name="vp", bufs=2) as vp_pool,
            tc.tile_pool(name="pt", bufs=2) as pt_pool,
            tc.tile_pool(name="onm", bufs=2) as onm_pool,
            tc.tile_pool(name="otp", bufs=3) as ot_pool,
            tc.tile_pool(name="outst", bufs=4) as outst_pool,
            tc.tile_pool(name="small", bufs=4) as small,
            tc.tile_pool(name="ps_sc", bufs=2, space="PSUM") as ps_sc,
            tc.tile_pool(name="ps_po", bufs=2, space="PSUM") as ps_po,
            tc.tile_pool(name="ps_mm", bufs=2, space="PSUM") as ps_mm,
        ):
            state = {}

            # ---- warmups: hold PE busy from t=0 so the p-state ramp is hot
            # by the time real operands arrive ----
            wz = nc.const_aps.tensor(1.0, (P, 256), bf16)
            for w in range(N_WARM):
                wm = ps_mm.tile([P, 512], f32, tag="mm", name="wm")
                _lab("warm"); nc.tensor.matmul(wm[:, 0:256], wz[:, 0:128], wz, start=True, stop=True)

            warm = consts.tile([P, 1], f32, tag="warm", name="warm")
            nc.vector.memset(warm, 0.0)
            nc.scalar.activation(out=warm, in_=warm, func=EXP, scale=1.0)

            ones16 = consts.tile([P, H], bf16, tag="ones16", name="ones16")
            nc.vector.memset(ones16, 16.0)

            walls = {}

            def load_w(wi, W):
                wt = consts.tile([P, NSLOT, D], fp8, tag=f"w{wi}",
                                 name=f"w{wi}")
                nc.sync.dma_start(
                    out=wt, in_=W[:].rearrange("s p e -> p s e"))
                walls[wi] = wt

            bo_row = consts.tile([1, D], bf16, tag="bo_row", name="bo_row")
            ones1 = consts.tile([1, P], bf16, tag="ones1", name="ones1")
            nc.vector.memset(ones1, 1.0)
            bo_bc = consts.tile([P, D], bf16, tag="bo_bc", name="bo_bc")

            wo_t = consts.tile([P, NDC, D], bf16, tag="wo", name="wo")

            # ---- DMA emitters ----
            def dma_xq(b):
                t = xin_pool.tile([P, NSLOT, N], fp8, tag="xq8", name="xq8")
                nc.sync.dma_start(
                    out=t, in_=xq8_d[b].rearrange("s p n -> p s n"))
                state[(b, "xq")] = t

            def dma_xk(b):
                lc = chunks[b] * P
                t = xin_pool.tile([P, NSLOT, lc], fp8, tag="xk8", name="xk8")
                nc.sync.dma_start(
                    out=t,
                    in_=xk8_d[b, :, :, 0:lc].rearrange("s p l -> p s l"))
                mbt = small.tile([P, chunks[b]], f32, tag="mbias", name="mbt")
                nc.sync.dma_start(
                    out=mbt, in_=mb_d[b, 0:lc].rearrange("(i p) -> p i", p=P))
                r = xin_pool.tile([P, NDC, lc], bf16, tag="rpbT", name="rpbT")
                nc.sync.dma_start(
                    out=r,
                    in_=rpbT_d[b, :, 0:lc].rearrange("(k p) l -> p k l", p=P))
                state[(b, "xk")] = t
                state[(b, "rpb")] = r
                state[(b, "mb")] = mbt

            # ---- prep compute groups (filler units) ----
            # fp8 DoubleRow 3-term projection: 6 insts per 128-col stationary
            def _proj3(dst, wt, wcols, xt, xcols, lab):
                k = 0
                for (xs, ws) in TERMS:
                    for s in range(2):
                        _lab(lab)
                        nc.tensor.matmul(
                            dst, wt[:, ws + 2 * s:ws + 2 * s + 2, wcols],
                            xt[:, xs + 2 * s:xs + 2 * s + 2, xcols],
                            start=(k == 0), stop=(k == 5), perf_mode=DR)
                        k += 1

            def qp_group(b, j):
                xq = state[(b, "xq")]
                pq = ps_mm.tile([P, N], f32, tag="mm", name="pq")
                _proj3(pq, walls[0], slice(j * P, (j + 1) * P),
                       xq, slice(0, N), f"QP b{b} j{j}")
                qt = qk_pool.tile([P, N], bf16, tag=f"qT{j}", name=f"qT{j}")
                nc.vector.tensor_copy(qt, pq)
                state.setdefault((b, "qT"), [None] * NDC)[j] = qt

            def kp_group(b, j, raw=False):
                lc = chunks[b] * P
                xk = state[(b, "xk")]
                rp = state[(b, "rpb")]
                dst = (qk_pool.tile([P, lc], bf16, tag=f"kraw{j}", name=f"kraw{j}")
                       if raw else
                       qk_pool.tile([P, lc], bf16, tag=f"kT{j}", name=f"kT{j}"))
                for (s0, s1) in _spans(lc):
                    pk = ps_mm.tile([P, N], f32, tag="mm", name="pk")
                    _proj3(pk[:, 0:s1 - s0], walls[1],
                           slice(j * P, (j + 1) * P),
                           xk, slice(s0, s1), f"KP b{b} j{j}")
                    if raw:
                        nc.vector.tensor_copy(dst[:, s0:s1], pk[:, 0:s1 - s0])
                    else:
                        nc.vector.tensor_add(dst[:, s0:s1], pk[:, 0:s1 - s0],
                                             rp[:, j, s0:s1])
                if raw:
                    state.setdefault((b, "kraw"), [None] * NDC)[j] = dst
                else:
                    state.setdefault((b, "kT"), [None] * NDC)[j] = dst

            def k_add(b, j):
                lc = chunks[b] * P
                rp = state[(b, "rpb")]
                kraw = state[(b, "kraw")][j]
                kt = qk_pool.tile([P, lc], bf16, tag=f"kT{j}", name=f"kT{j}")
                nc.vector.tensor_add(kt, kraw, rp[:, j, :])
                state.setdefault((b, "kT"), [None] * NDC)[j] = kt

            def vp_group(b, i):
                xk = state[(b, "xk")]
                pv = ps_mm.tile([P, N], f32, tag="mm", name="pv")
                _proj3(pv, xk, slice(i * P, (i + 1) * P),
                       walls[2], slice(0, D), f"VP b{b} i{i}")
                t = vp_pool.tile([P, H, C + 1], bf16, tag=f"vp{i}",
                                 name=f"vp{i}")
                nc.vector.tensor_copy(
                    t[:, :, 0:C], pv.rearrange("p (h c) -> p h c", h=H))
                nc.gpsimd.tensor_copy(t[:, :, C:C + 1], ones16[:, :, None])
                state.setdefault((b, "vP"), [None] * cmax)[i] = t

            def op_group(b, m):
                oT = state[(b, "oT")]
                pf = ps_mm.tile([P, N], f32, tag="mm", name="pf")
                for j in range(NDC):
                    _lab(f"OP b{b} m{m}"); nc.tensor.matmul(pf, oT[j][:, m * P:(m + 1) * P],
                                     wo_t[:, j, :],
                                     start=(j == 0), stop=(j == NDC - 1))
                to = outst_pool.tile([P, D], bf16, tag="outst", name="to")
                nc.vector.tensor_add(to, pf, bo_bc)
                nc.sync.dma_start(out=out[b, m * P:(m + 1) * P, :], in_=to)

            # last batch: accumulate the j<3 o_proj terms + bias into the sc
            # psum ring (dead after the last exp) while pair (last, j3)
            # winds down, leaving only the j3 term + copy + DMA for the tail
            parts = {}

            def op_part(b, m):
                if m % 2 == 0:
                    pw = ps_sc.tile([P, 2 * N], f32, tag="sc", name="pw")
                    parts[m] = pw[:, 0:N]
                    parts[m + 1] = pw[:, N:2 * N]
                pp = parts[m]
                oT = state[(b, "oT")]
                for j in range(NDC - 1):
                    _lab(f"OPp b{b} m{m}"); nc.tensor.matmul(pp, oT[j][:, m * P:(m + 1) * P],
                                     wo_t[:, j, :],
                                     start=(j == 0), stop=False,
                                     skip_group_check=True)
                _lab(f"OPb b{b} m{m}")
                nc.tensor.matmul(pp, ones1, bo_row, start=False, stop=False,
                                 skip_group_check=True)

            # ---- attention pair ----
            def pair(b, j, fillers, tail_fillers=()):
                c = chunks[b]
                last = b == BLOC - 1 and j == NDC - 1
                mbt = state[(b, "mb")]
                qT, kT = state[(b, "qT")], state[(b, "kT")]
                vP = state[(b, "vP")]
                po = {}
                for half in range(2):
                    po[half] = ps_po.tile([P, NNC, C + 1], f32, tag="po",
                                          name="po")
                pes = []
                fi = 0

                # PV interleaved into the S/exp stream: as soon as exp(i) is
                # done, its l-chunk is accumulated into every (m, half) psum
                # region, so PV never waits for the full exp stream.
                def pv_i(i, on_m=None):
                    pe = pes[i]
                    for m in range(NNC):
                        for half in range(2):
                            stat = pe[:, half * N + m * P:
                                       half * N + (m + 1) * P]
                            _lab(f"PV b{b} j{j} i{i} m{m}")
                            nc.tensor.matmul(
                                po[half][:, m, :], stat,
                                vP[i][:, 2 * j + half, :],
                                # first write per po bank: start=True marks
                                # the whole 2KB bank zero-on-write, replacing
                                # an explicit DVE memset
                                start=(i == 0 and m == 0),
                                stop=(i == c - 1),
                                skip_group_check=True)
                        if on_m is not None:
                            on_m(m)

                def s_exp(i):
                    pss = ps_sc.tile([P, 2 * N], f32, tag="sc", name="pss")
                    for half in range(2):
                        lo = C * half
                        _lab(f"S b{b} j{j} i{i}")
                        nc.tensor.matmul(
                            pss[:, half * N:(half + 1) * N],
                            kT[j][lo:lo + C, i * P:(i + 1) * P],
                            qT[j][lo:lo + C, :], start=True, stop=True,
                            tile_position=(lo, 0))
                    pe = pt_pool.tile([P, 2 * N], bf16, tag=f"pe{i}",
                                      name=f"pe{i}")
                    nc.scalar.activation(out=pe, in_=pss, func=EXP,
                                         bias=mbt[:, i:i + 1], scale=SSCALE)
                    pes.append(pe)

                # lag PV two rounds behind S/exp so the ACT stream is never
                # awaited (exp(i-2) is safely done when pv_i(i-2) issues)
                for i in range(c):
                    s_exp(i)
                    if i >= 1:
                        if i >= 3:
                            pv_i(i - 3)
                        if fi < len(fillers):
                            fillers[fi]()
                            fi += 1
                for k in (3, 2):
                    if c >= k:
                        pv_i(c - k)

                tr = ps_mm.tile([P, N], bf16, tag="mm", name="tr")
                onm = []

                def norm_m(m):
                    o = onm_pool.tile([P, P], bf16, tag=f"on{m}",
                                      name=f"on{m}")
                    for half in range(2):
                        r = small.tile([P, 1], f32, tag=f"rcp{half}",
                                       name=f"r{half}")
                        nc.vector.reciprocal(r, po[half][:, m, C:C + 1])
                        nc.vector.tensor_scalar_mul(
                            o[:, half * C:(half + 1) * C],
                            po[half][:, m, 0:C], r)
                    onm.append(o)

                # leftover fillers buy time for exp(c-1) to land before the
                # final PV round needs it
                while fi < len(fillers):
                    fillers[fi]()
                    fi += 1
                ot = ot_pool.tile([P, N], bf16, tag=f"oT{j}", name=f"oT{j}")
                if last:
                    # wind-down: the whole per-m output chain (o_proj partial
                    # -> transpose -> ot slice -> j3 term -> copy -> DMA)
                    # fires as each m region closes; the j3 term trails one m
                    # behind so the ot-slice latency hides under the next m's
                    # partial o_proj
                    def opf_m(m):
                        sl = slice(m * P, (m + 1) * P)
                        _lab(f"OPf b{b} m{m}")
                        nc.tensor.matmul(parts[m], ot[:, sl],
                                         wo_t[:, NDC - 1, :], start=False,
                                         stop=True, skip_group_check=True)
                        to = outst_pool.tile([P, D], bf16, tag="outst2",
                                             name="to2")
                        if m % 2 == 0:
                            nc.scalar.copy(to, parts[m])
                        else:
                            nc.vector.tensor_copy(to, parts[m])
                        nc.sync.dma_start(out=out[b, m * P:(m + 1) * P, :],
                                          in_=to)

                    def final_m(m):
                        norm_m(m)
                        op_part(b, m)
                        _lab(f"T b{b} j{j} m{m}")
                        nc.tensor.transpose(
                            tr[:, m * P:(m + 1) * P], onm[m], ident)
                        sl = slice(m * P, (m + 1) * P)
                        if m % 2 == 0:
                            nc.scalar.copy(ot[:, sl], tr[:, sl])
                        else:
                            nc.vector.tensor_copy(ot[:, sl], tr[:, sl])
                        if m >= 1:
                            opf_m(m - 1)
                    pv_i(c - 1, on_m=final_m)
                    opf_m(NNC - 1)
                else:
                    # last l-chunk: per-m norms fire as each region closes
                    pv_i(c - 1, on_m=norm_m)
                    for m in range(NNC):
                        _lab(f"T b{b} j{j} m{m}")
                        nc.tensor.transpose(
                            tr[:, m * P:(m + 1) * P], onm[m], ident)
                    nc.vector.tensor_copy(ot, tr)
                state.setdefault((b, "oT"), [None] * NDC)[j] = ot

            # ---- schedule ----
            ident = consts.tile([P, P], bf16, tag="ident", name="ident")
            load_w(0, Wq8)
            dma_xq(0)
            load_w(1, Wk8)
            # xk + mb first, Wv before rpb so VP unblocks early, rpb gated last
            lc0 = chunks[0] * P
            t0 = xin_pool.tile([P, NSLOT, lc0], fp8, tag="xk8", name="xk8")
            nc.sync.dma_start(
                out=t0, in_=xk8_d[0, :, :, 0:lc0].rearrange("s p l -> p s l"))
            mbt0 = small.tile([P, chunks[0]], f32, tag="mbias", name="mbt")
            nc.sync.dma_start(
                out=mbt0, in_=mb_d[0, 0:lc0].rearrange("(i p) -> p i", p=P))
            load_w(2, Wv8)
            r0 = xin_pool.tile([P, NDC, lc0], bf16, tag="rpbT", name="rpbT")
            for k in range(NDC):
                nc.sync.dma_start(
                    out=r0[:, k, :], in_=rpbT_d[0, k * P:(k + 1) * P, 0:lc0])
            state[(0, "xk")] = t0
            state[(0, "rpb")] = r0
            state[(0, "mb")] = mbt0
            nc.sync.dma_start(
                out=wo_t, in_=Wo[:].rearrange("(k p) e -> p k e", p=P))
            nc.sync.dma_start(out=ident, in_=id_d[:])
            nc.sync.dma_start(out=bo_row, in_=bo[:])
            nc.gpsimd.partition_broadcast(bo_bc, bo_row[0:1, :], channels=P)
            for j in range(NDC):
                qp_group(0, j)
            for j in range(NDC):
                kp_group(0, j, raw=True)
            vp_group(0, 0)
            vp_group(0, 1)
            for j in range(NDC):
                k_add(0, j)

            def F(fn, *a):
                return lambda: fn(*a)

            for b in range(BLOC):
                nxt = b + 1 < BLOC
                for j in range(NDC):
                    fillers = []
                    if j == 0:
                        fillers += [F(vp_group, b, i)
                                    for i in range(2, chunks[b])]
                        if b > 0:
                            fillers.append(F(kp_group, b, 2))
                        if b > 1:
                            fillers.append(F(op_group, b - 2, 2))
                        if nxt:
                            dma_xq(b + 1)
                            dma_xk(b + 1)
                    elif j == 1:
                        if b > 0:
                            fillers.append(F(kp_group, b, 3))
                            fillers.append(F(op_group, b - 1, 0))
                        if b > 1:
                            fillers.append(F(op_group, b - 2, 3))
                        if nxt:
                            fillers += [F(qp_group, b + 1, 0),
                                        F(qp_group, b + 1, 1)]
                    elif j == 2:
                        if nxt:
                            fillers += [F(qp_group, b + 1, 2),
                                        F(qp_group, b + 1, 3),
                                        F(vp_group, b + 1, 0),
                                        F(vp_group, b + 1, 1)]
                        if not nxt:
                            fillers += [F(op_group, b - 1, 1),
                                        F(op_group, b - 1, 2)]
                    elif j == 3:
                        if nxt:
                            fillers += [F(kp_group, b + 1, 0),
                                        F(kp_group, b + 1, 1)]
                            if b > 0:
                                fillers.append(F(op_group, b - 1, 1))
                    if not nxt and j == 3:
                        fillers.append(F(op_group, b - 1, 3))
                    pair(b, j, fillers)

    nc.compile()
    return nc


def _get_nc(chunks=(4, 5, 5, 5)):
    chunks = tuple(chunks)
    if chunks not in _CACHE:
        _CACHE[chunks] = _build_nc(chunks)
    return _CACHE[chunks]


def _split8(a, e4):
    """f32 array -> (hi, lo) fp8 e4m3 pair with hi + lo ~= a."""
    hi = a.astype(e4)
    lo = (a - hi.astype(np.float32)).astype(e4)
    return hi, lo


def kernel(x_q, x_kv, pad_mask, Wq, Wk, Wv, Wo, bo, rpb):
    from concourse.bass_utils import run_bass_kernel_spmd
    import ml_dtypes

    bf = ml_dtypes.bfloat16
    e4 = ml_dtypes.float8_e4m3fn
    x_q = np.asarray(x_q, dtype=np.float32)
    x_kv = np.asarray(x_kv, dtype=np.float32)
    pad_mask = np.asarray(pad_mask).astype(bool)
    rpb2 = np.asarray(rpb, np.float32).reshape(L, D)

    counts = (~pad_mask).sum(axis=1)
    # ascending: the smallest batch goes first (startup is DMA-gated, so a
    # cheap batch 0 wastes less PE) and big batches fill the tail
    order = np.argsort(counts, kind="stable")  # rank -> global batch
    # slot s of core c processes global batch order[s*NCORES + c]
    chunks = []
    for s in range(BLOC):
        grp = counts[order[s * NCORES:(s + 1) * NCORES]]
        chunks.append(max(1, -(-int(grp.max()) // P)))
    chunks = tuple(chunks)
    nc = _get_nc(chunks)

    def wsplit(W):
        W16 = 16.0 * np.asarray(W, np.float32)
        hi, lo = _split8(W16, e4)
        o = np.zeros((NSLOT, P, D), e4)
        for c in range(NDC):
            o[c] = hi[c * P:(c + 1) * P]
            o[NDC + c] = lo[c * P:(c + 1) * P]
        return o

    shared = {
        "Wq8": wsplit(Wq),
        "Wk8": wsplit(Wk),
        "Wv8": wsplit(Wv),
        "Wo": np.asarray(Wo, np.float32).astype(bf),
        "bo": np.asarray(bo, np.float32).reshape(1, D).astype(bf),
        "ident": np.eye(P, dtype=np.float32).astype(bf),
    }
    in_maps = []
    for c in range(NCORES):
        xq8 = np.zeros((BLOC, NSLOT, P, N), e4)
        xk8 = np.zeros((BLOC, NSLOT, P, LCMAX), e4)
        rpbT = np.zeros((BLOC, D, LCMAX), bf)
        mb = np.full((BLOC, LCMAX), MASK_NEG, np.float32)
        for s in range(BLOC):
            g = order[s * NCORES + c]
            idx = np.nonzero(~pad_mask[g])[0]
            cnt = len(idx)
            xqT = x_q[g].T  # [D, N] f32
            hi, lo = _split8(xqT, e4)
            for k in range(NDC):
                xq8[s, k] = hi[k * P:(k + 1) * P]
                xq8[s, NDC + k] = lo[k * P:(k + 1) * P]
            xkT = x_kv[g, idx, :].T  # [D, cnt] f32
            hi, lo = _split8(xkT, e4)
            for k in range(NDC):
                xk8[s, k, :, :cnt] = hi[k * P:(k + 1) * P]
                xk8[s, NDC + k, :, :cnt] = lo[k * P:(k + 1) * P]
            rpbT[s, :, :cnt] = (16.0 * rpb2[idx, :].T).astype(bf)
            mb[s, :cnt] = 0.0
        in_maps.append({
            "xq8": xq8, "xk8": xk8, "rpbT": rpbT, "mbias": mb, **shared,
        })
    res = run_bass_kernel_spmd(nc, in_maps, list(range(NCORES)))
    outp = np.empty((B, N, D), np.float32)
    for c in range(NCORES):
        for s in range(BLOC):
            outp[order[s * NCORES + c]] = np.asarray(
                res.results[c]["out"][s], dtype=np.float32)
    return outp
